# revision 3
# baseline (speedup 1.0000x reference)
"""Optimized Trainium2 Bass kernel v4: GCN message passing, 8 cores SPMD.

v6 -> v7: epilogue delayed TWO groups behind the matmuls (pend queue)
so DVE never has to wait for PE/gathers before issuing the next group's
S windows; psum_bufs=4 keeps three accumulation generations alive;
deeper S-window and meta buffering.
"""

import numpy as np

import concourse.bass as bass
import concourse.bacc as bacc
import concourse.tile as tile
import concourse.mybir as mybir
from concourse.bass_utils import run_bass_kernel_spmd

MED = 50000
NCORES = 8
TILES = 49               # dest tiles per plane per core
RPC = TILES * 128        # 6272 dest rows per core (per plane)
HALF = 25000             # balanced table split (fits int16)
P = 128
F = 128
NQ = 4                   # SWDGE queues

_NC_CACHE = {}


def build_nc(C0, C1, tiles=TILES, gbufs=4, repeat=1, nq=NQ, psum_bufs=4,
             scratch=16384, gsplit=2, mode="full", epdelay=2, spbufs=12):
    """C0/C1: chunks per half0/half1 run. Group chunk layout: [A0|B0|A1|B1]
    (A=plane0, B=plane1; 0/1 = table half). One gather call per run, queues
    rotate per group."""
    CG = 2 * C0 + 2 * C1
    IC = CG * 8              # idx int16 cols (num_idxs/16 per chunk = 8)
    dt16 = mybir.dt.float16
    f32 = mybir.dt.float32

    nc = bacc.Bacc(None, target_bir_lowering=False, num_swdge_queues=nq,
                   dynamic_dma_scratch_size=scratch)
    table = nc.dram_tensor("table", [MED, F], dt16, kind="ExternalInput")
    idx_d = nc.dram_tensor("idx", [tiles, P, IC], mybir.dt.int16, kind="ExternalInput")
    dval_d = nc.dram_tensor("dval", [tiles, P, 2 * CG], dt16, kind="ExternalInput")
    ab_d = nc.dram_tensor("ab", [P, 2], f32, kind="ExternalInput")
    Wmax = max(C0, C1)
    iota_d = nc.dram_tensor("iota", [P, P * Wmax], dt16, kind="ExternalInput")
    out_d = nc.dram_tensor("out", [tiles, P, F], f32, kind="ExternalOutput")

    planes = [0] * C0 + [1] * C0 + [0] * C1 + [1] * C1
    # runs: (chunk_lo, chunk_hi, table_lo, table_hi)
    runs = [
        (0, C0, 0, HALF),
        (C0, 2 * C0, 0, HALF),
        (2 * C0, 2 * C0 + C1, HALF, MED),
        (2 * C0 + C1, CG, HALF, MED),
    ]
    firstA, lastA = 0, 2 * C0 + C1 - 1
    firstB, lastB = C0, CG - 1

    with tile.TileContext(nc) as tc:
        with (
            tc.tile_pool(name="const", bufs=1) as constp,
            tc.tile_pool(name="gbuf", bufs=gbufs) as gbufp,
            tc.tile_pool(name="meta", bufs=5) as metap,
            tc.tile_pool(name="sp", bufs=spbufs) as sp,
            tc.tile_pool(name="ep", bufs=4) as ep,
            tc.tile_pool(name="psum", bufs=psum_bufs, space=bass.MemorySpace.PSUM) as psp,
        ):
            # chunk-minor iota: iota_cm[p, r, w] = r
            iota_t = constp.tile([P, P, Wmax], dt16, tag="iota")
            nc.sync.dma_start(iota_t[:].rearrange("p r w -> p (r w)"), iota_d[:])
            ab_t = constp.tile([P, 2], f32, tag="ab")
            nc.sync.dma_start(ab_t[:], ab_d[:])

            def epilogue(psA, psB, g):
                t0 = ep.tile([P, F], f32, tag="t0")
                nc.vector.tensor_scalar(t0[:], psA[:], 0.0, ab_t[:, 0:1],
                                        mybir.AluOpType.max, mybir.AluOpType.mult)
                t1 = ep.tile([P, F], f32, tag="t1")
                nc.vector.tensor_scalar(t1[:], psB[:], 0.0, ab_t[:, 1:2],
                                        mybir.AluOpType.max, mybir.AluOpType.mult)
                o_t = ep.tile([P, F], f32, tag="o")
                nc.vector.tensor_tensor(o_t[:], t0[:], t1[:], mybir.AluOpType.add)
                nc.scalar.dma_start(out_d[g], o_t[:])

            gi = 0
            pend = []
            for g in [g_ for _ in range(repeat) for g_ in range(tiles)]:
                idx_t = metap.tile([P, IC], mybir.dt.int16, tag="idx")
                nc.sync.dma_start(idx_t[:], idx_d[g])
                dv_t = metap.tile([P, 1, 2 * CG], dt16, tag="dval")
                nc.sync.dma_start(dv_t[:, 0, :], dval_d[g])

                g_t = gbufp.tile([P, CG, F], dt16, tag="g")
                if mode != "compute_only":
                    for (lo0, hi0, tlo, thi) in runs:
                        step = max(1, (hi0 - lo0 + gsplit - 1) // gsplit)
                        for lo in range(lo0, hi0, step):
                            hi = min(lo + step, hi0)
                            n = (hi - lo) * 128
                            nc.gpsimd.dma_gather(
                                g_t[:, lo:hi, :], table[tlo:thi, :],
                                idx_t[:, lo * 8:hi * 8], n, n, F,
                                single_packet=False,
                                queue_num=gi % nq,
                            )
                            gi += 1
                else:
                    nc.sync.dma_start(
                        g_t[:].rearrange("p c f -> p (c f)"),
                        table[0:CG, :].rearrange("c f -> (c f)")
                        .to_broadcast([P, CG * F]))

                if mode == "gather_only":
                    o_t = ep.tile([P, F], f32, tag="o")
                    nc.vector.tensor_copy(o_t[:], g_t[:, 0, :])
                    nc.vector.tensor_add(o_t[:], o_t[:], g_t[:, CG - 1, :])
                    nc.scalar.dma_start(out_d[g], o_t[:])
                    continue

                # per-run windowed S build, chunk-minor: two packed DVE ops
                s_ts = []
                for (lo, hi, _, _) in runs:
                    W = hi - lo
                    s_t = sp.tile([P, P, W], dt16, tag="s")
                    nc.vector.tensor_tensor(
                        s_t[:], iota_t[:, :, 0:W],
                        dv_t[:, 0:1, lo:hi].to_broadcast([P, P, W]),
                        mybir.AluOpType.is_equal)
                    nc.vector.tensor_tensor(
                        s_t[:], s_t[:],
                        dv_t[:, 0:1, CG + lo:CG + hi].to_broadcast([P, P, W]),
                        mybir.AluOpType.mult)
                    s_ts.append((lo, s_t))

                def s_ap(c):
                    for lo, s_t in reversed(s_ts):
                        if c >= lo:
                            return s_t[:, :, c - lo]
                    raise AssertionError

                # delayed epilogues: DVE never waits on recent PE output
                if len(pend) >= epdelay:
                    epilogue(*pend.pop(0))

                psA = psp.tile([P, F], f32, tag="psA")
                psB = psp.tile([P, F], f32, tag="psB")
                for c in range(CG):
                    if planes[c] == 0:
                        nc.tensor.matmul(psA[:], s_ap(c), g_t[:, c, :],
                                         start=(c == firstA), stop=(c == lastA))
                    else:
                        nc.tensor.matmul(psB[:], s_ap(c), g_t[:, c, :],
                                         start=(c == firstB), stop=(c == lastB))
                pend.append((psA, psB, g))
            for args in pend:
                epilogue(*args)

    nc.compile()
    return nc


def preprocess(vals, mEmbed, inter, row_idx, col_idx, tiles=TILES):
    E = row_idx.shape[0]
    col = col_idx.astype(np.int64) % MED
    rowl = row_idx.astype(np.int64)
    plane = rowl // MED
    prow = rowl % MED
    core = np.minimum(prow // RPC, NCORES - 1)
    lt = (prow - core * RPC) >> 7
    d = (prow & 127).astype(np.float32)
    half = (col >= HALF).astype(np.int64)
    lidx = (col - half * HALF).astype(np.int16)

    run = half * 2 + plane                      # A0,B0,A1,B1 order
    key = (core * tiles + lt) * 4 + run
    order = np.argsort(key, kind="stable")
    ksort = key[order]
    nk = NCORES * tiles * 4
    cnt = np.bincount(ksort, minlength=nk)
    starts = np.concatenate([[0], np.cumsum(cnt)[:-1]])
    rank = np.arange(E, dtype=np.int64) - starts[ksort]

    cnt4 = cnt.reshape(-1, 4)
    C0 = max(1, int(np.ceil(cnt4[:, 0:2].max() / 128)))
    C1 = max(1, int(np.ceil(cnt4[:, 2:4].max() / 128)))
    CG = 2 * C0 + 2 * C1
    run_off = np.array([0, C0 * 128, 2 * C0 * 128, (2 * C0 + C1) * 128])
    SLOTS_G = CG * 128
    gidx = ksort // 4
    slot = gidx * SLOTS_G + run_off[ksort % 4] + rank
    TOT = NCORES * tiles * SLOTS_G

    IDX = np.zeros(TOT, np.int16)
    VAL = np.zeros(TOT, np.float32)
    DD = np.zeros(TOT, np.float32)
    IDX[slot] = lidx[order]
    VAL[slot] = np.asarray(vals, np.float32)[order]
    DD[slot] = d[order]

    # idx layout per chunk: 128 idxs wrapped into 16 partitions -> [16, 8],
    # replicated to 128 partitions; chunks side by side -> [128, CG*8]
    IDX5 = IDX.reshape(NCORES, tiles, CG, 8, 16).transpose(0, 1, 4, 2, 3)
    idx16 = IDX5.reshape(NCORES, tiles, 16, CG * 8)
    idx128 = np.ascontiguousarray(np.tile(idx16, (1, 1, 8, 1)))

    D4 = DD.reshape(NCORES, tiles, CG, 128).transpose(0, 1, 3, 2)
    V4 = VAL.reshape(NCORES, tiles, CG, 128).transpose(0, 1, 3, 2)
    dval = np.ascontiguousarray(np.concatenate([D4, V4], axis=3), dtype=np.float16)

    table16 = np.asarray(mEmbed, np.float32).astype(np.float16)
    Wmax = max(C0, C1)
    iota = np.ascontiguousarray(np.broadcast_to(
        np.repeat(np.arange(128, dtype=np.float16), Wmax)[None, :],
        (128, 128 * Wmax)))
    a = 2.0 * np.float32(np.asarray(inter).reshape(-1)[0])
    b = np.float32(2.0) - a
    ab = np.ascontiguousarray(
        np.stack([np.full(128, a, np.float32), np.full(128, b, np.float32)], axis=1))
    return C0, C1, table16, iota, ab, idx128, dval


def _run(vals, mEmbed, inter, row_idx, col_idx, trace=False):
    C0, C1, table16, iota, ab, idx128, dval = preprocess(
        vals, mEmbed, inter, row_idx, col_idx)
    key = (C0, C1, 1, ())
    if key not in _NC_CACHE:
        _NC_CACHE[key] = build_nc(C0, C1)
    nc = _NC_CACHE[key]
    in_maps = [
        {"table": table16, "iota": iota, "ab": ab,
         "idx": idx128[k], "dval": dval[k]}
        for k in range(NCORES)
    ]
    res = run_bass_kernel_spmd(nc, in_maps, core_ids=list(range(NCORES)),
                               trace=trace)
    full = np.concatenate(
        [res.results[k]["out"].reshape(RPC, F) for k in range(NCORES)], axis=0)
    return np.ascontiguousarray(full[:MED]), res


def kernel(vals, mEmbed, inter, row_idx, col_idx):
    out, _ = _run(vals, mEmbed, inter, row_idx, col_idx, trace=False)
    return out


def _make_sharded(nc, donate=False):
    import jax
    from jax.sharding import Mesh, PartitionSpec
    from jax.experimental.shard_map import shard_map
    from concourse import bass2jax as b2j

    b2j.install_neuronx_cc_hook()
    partition_name = nc.partition_id_tensor.name if nc.partition_id_tensor else None
    in_names, out_names, out_avals, zero_outs = [], [], [], []
    for alloc in nc.m.functions[0].allocations:
        if not isinstance(alloc, mybir.MemoryLocationSet):
            continue
        name = alloc.memorylocations[0].name
        if alloc.kind == "ExternalInput":
            if name != partition_name:
                in_names.append(name)
        elif alloc.kind == "ExternalOutput":
            out_names.append(name)
            shape = tuple(alloc.tensor_shape)
            dtype = mybir.dt.np(alloc.dtype)
            out_avals.append(jax.core.ShapedArray(shape, dtype))
            zero_outs.append(np.zeros(shape, dtype))
    n_params = len(in_names)
    in_names = in_names + out_names
    if partition_name is not None:
        in_names = in_names + [partition_name]

    def _body(*args):
        operands = list(args)
        if partition_name is not None:
            operands.append(b2j.partition_id_tensor())
        outs = b2j._bass_exec_p.bind(
            *operands,
            out_avals=tuple(out_avals),
            in_names=tuple(in_names),
            out_names=tuple(out_names),
            lowering_input_output_aliases=(),
            sim_require_finite=True,
            sim_require_nnan=True,
            nc=nc,
        )
        return tuple(outs)

    devices = jax.devices()[:NCORES]
    mesh = Mesh(np.asarray(devices), ("core",))
    in_specs = (PartitionSpec("core"),) * (n_params + len(out_names))
    out_specs = (PartitionSpec("core"),) * len(out_names)
    kw = dict(donate_argnums=tuple(range(n_params, n_params + len(out_names)))) if donate else {}

    sharded = jax.jit(
        shard_map(_body, mesh=mesh, in_specs=in_specs,
                  out_specs=out_specs, check_rep=False),
        keep_unused=True, **kw)
    return sharded, mesh, in_names[:n_params], out_names, zero_outs


def timed_run(vals, mEmbed, inter, row_idx, col_idx, k=4, samples=5,
              build_kwargs=None):
    """Time on device: build the same program with the body repeated 1x and
    kx INSIDE the NEFF; marginal = (T(k) - T(1)) / (k-1) = pure HW time."""
    import time
    import jax
    from jax.sharding import NamedSharding, PartitionSpec

    C0, C1, table16, iota, ab, idx128, dval = preprocess(
        vals, mEmbed, inter, row_idx, col_idx)
    bk = dict(build_kwargs or {})
    per_core = [
        {"table": table16, "iota": iota, "ab": ab,
         "idx": idx128[k_], "dval": dval[k_]}
        for k_ in range(NCORES)
    ]

    def measure(repeat):
        ck = (C0, C1, repeat, tuple(sorted(bk.items())))
        if ck not in _NC_CACHE:
            _NC_CACHE[ck] = build_nc(C0, C1, repeat=repeat, **bk)
        nc = _NC_CACHE[ck]
        sharded, mesh, in_names, out_names, zero_outs = _make_sharded(nc)
        sh = NamedSharding(mesh, PartitionSpec("core"))
        concat_in = [
            jax.device_put(
                np.concatenate([np.asarray(per_core[c][n]) for c in range(NCORES)],
                               axis=0), sh)
            for n in in_names
        ]
        concat_zero = [
            jax.device_put(np.zeros((NCORES * z.shape[0], *z.shape[1:]), z.dtype), sh)
            for z in zero_outs
        ]
        out = sharded(*concat_in, *concat_zero)
        jax.block_until_ready(out)
        best = float("inf")
        for _ in range(samples):
            t0 = time.perf_counter()
            out = sharded(*concat_in, *concat_zero)
            jax.block_until_ready(out)
            best = min(best, time.perf_counter() - t0)
        return best

    t1 = measure(1)
    tk = measure(k)
    marginal_ns = (tk - t1) / (k - 1) * 1e9
    return int(marginal_ns), int(t1 * 1e9), int(tk * 1e9)


# revision 5
# speedup vs baseline: 1.1013x; 1.1013x over previous
"""Optimized Trainium2 Bass kernel v4: GCN message passing, 8 cores SPMD.

v6 -> v7: epilogue delayed TWO groups behind the matmuls (pend queue)
so DVE never has to wait for PE/gathers before issuing the next group's
S windows; psum_bufs=4 keeps three accumulation generations alive;
deeper S-window and meta buffering.
"""

import numpy as np

import concourse.bass as bass
import concourse.bacc as bacc
import concourse.tile as tile
import concourse.mybir as mybir
from concourse.bass_utils import run_bass_kernel_spmd

MED = 50000
NCORES = 8
TILES = 49               # dest tiles per plane per core
RPC = TILES * 128        # 6272 dest rows per core (per plane)
H0_HI = 32768            # half0 covers table rows [0, 32768)
FLEX_LO = MED - 32768    # half1 covers rows [17232, 50000); overlap is flexible
P = 128
F = 128
NQ = 4                   # SWDGE queues

_NC_CACHE = {}


def build_nc(C0, C1, tiles=TILES, gbufs=6, repeat=1, nq=NQ, psum_bufs=4,
             scratch=16384, gsplit=2, mode="full", epdelay=3, spbufs=16):
    """C0/C1: chunks per half0/half1 run. Group chunk layout: [A0|B0|A1|B1]
    (A=plane0, B=plane1; 0/1 = table half). One gather call per run, queues
    rotate per group."""
    CG = 2 * C0 + 2 * C1
    IC = CG * 8              # idx int16 cols (num_idxs/16 per chunk = 8)
    dt16 = mybir.dt.float16
    f32 = mybir.dt.float32

    nc = bacc.Bacc(None, target_bir_lowering=False, num_swdge_queues=nq,
                   dynamic_dma_scratch_size=scratch)
    table = nc.dram_tensor("table", [MED, F], dt16, kind="ExternalInput")
    idx_d = nc.dram_tensor("idx", [tiles, P, IC], mybir.dt.int16, kind="ExternalInput")
    dval_d = nc.dram_tensor("dval", [tiles, P, 2 * CG], dt16, kind="ExternalInput")
    ab_d = nc.dram_tensor("ab", [P, 2], f32, kind="ExternalInput")
    Wmax = max(C0, C1)
    iota_d = nc.dram_tensor("iota", [P, P * Wmax], dt16, kind="ExternalInput")
    out_d = nc.dram_tensor("out", [tiles, P, F], f32, kind="ExternalOutput")

    planes = [0] * C0 + [1] * C0 + [0] * C1 + [1] * C1
    # runs: (chunk_lo, chunk_hi, table_lo, table_hi)
    runs = [
        (0, C0, 0, H0_HI),
        (C0, 2 * C0, 0, H0_HI),
        (2 * C0, 2 * C0 + C1, FLEX_LO, MED),
        (2 * C0 + C1, CG, FLEX_LO, MED),
    ]
    firstA, lastA = 0, 2 * C0 + C1 - 1
    firstB, lastB = C0, CG - 1

    with tile.TileContext(nc) as tc:
        with (
            tc.tile_pool(name="const", bufs=1) as constp,
            tc.tile_pool(name="gbuf", bufs=gbufs) as gbufp,
            tc.tile_pool(name="meta", bufs=7) as metap,
            tc.tile_pool(name="sp", bufs=spbufs) as sp,
            tc.tile_pool(name="ep", bufs=4) as ep,
            tc.tile_pool(name="psum", bufs=psum_bufs, space=bass.MemorySpace.PSUM) as psp,
        ):
            # chunk-minor iota: iota_cm[p, r, w] = r
            iota_t = constp.tile([P, P, Wmax], dt16, tag="iota")
            nc.sync.dma_start(iota_t[:].rearrange("p r w -> p (r w)"), iota_d[:])
            ab_t = constp.tile([P, 2], f32, tag="ab")
            nc.sync.dma_start(ab_t[:], ab_d[:])

            def epilogue(psA, psB, g):
                t0 = ep.tile([P, F], f32, tag="t0")
                nc.vector.tensor_scalar(t0[:], psA[:], 0.0, ab_t[:, 0:1],
                                        mybir.AluOpType.max, mybir.AluOpType.mult)
                t1 = ep.tile([P, F], f32, tag="t1")
                nc.vector.tensor_scalar(t1[:], psB[:], 0.0, ab_t[:, 1:2],
                                        mybir.AluOpType.max, mybir.AluOpType.mult)
                o_t = ep.tile([P, F], f32, tag="o")
                nc.vector.tensor_tensor(o_t[:], t0[:], t1[:], mybir.AluOpType.add)
                nc.scalar.dma_start(out_d[g], o_t[:])

            gi = 0
            pend = []
            for g in [g_ for _ in range(repeat) for g_ in range(tiles)]:
                idx_t = metap.tile([P, IC], mybir.dt.int16, tag="idx")
                nc.sync.dma_start(idx_t[:], idx_d[g])
                dv_t = metap.tile([P, 1, 2 * CG], dt16, tag="dval")
                nc.sync.dma_start(dv_t[:, 0, :], dval_d[g])

                g_t = gbufp.tile([P, CG, F], dt16, tag="g")
                if mode != "compute_only":
                    for (lo0, hi0, tlo, thi) in runs:
                        step = max(1, (hi0 - lo0 + gsplit - 1) // gsplit)
                        for lo in range(lo0, hi0, step):
                            hi = min(lo + step, hi0)
                            n = (hi - lo) * 128
                            nc.gpsimd.dma_gather(
                                g_t[:, lo:hi, :], table[tlo:thi, :],
                                idx_t[:, lo * 8:hi * 8], n, n, F,
                                single_packet=False,
                                queue_num=gi % nq,
                            )
                            gi += 1
                else:
                    nc.sync.dma_start(
                        g_t[:].rearrange("p c f -> p (c f)"),
                        table[0:CG, :].rearrange("c f -> (c f)")
                        .to_broadcast([P, CG * F]))

                if mode == "gather_only":
                    o_t = ep.tile([P, F], f32, tag="o")
                    nc.vector.tensor_copy(o_t[:], g_t[:, 0, :])
                    nc.vector.tensor_add(o_t[:], o_t[:], g_t[:, CG - 1, :])
                    nc.scalar.dma_start(out_d[g], o_t[:])
                    continue

                # per-run windowed S build, chunk-minor: two packed DVE ops
                s_ts = []
                for (lo, hi, _, _) in runs:
                    W = hi - lo
                    s_t = sp.tile([P, P, W], dt16, tag="s")
                    nc.vector.tensor_tensor(
                        s_t[:], iota_t[:, :, 0:W],
                        dv_t[:, 0:1, lo:hi].to_broadcast([P, P, W]),
                        mybir.AluOpType.is_equal)
                    nc.vector.tensor_tensor(
                        s_t[:], s_t[:],
                        dv_t[:, 0:1, CG + lo:CG + hi].to_broadcast([P, P, W]),
                        mybir.AluOpType.mult)
                    s_ts.append((lo, s_t))

                def s_ap(c):
                    for lo, s_t in reversed(s_ts):
                        if c >= lo:
                            return s_t[:, :, c - lo]
                    raise AssertionError

                # delayed epilogues: DVE never waits on recent PE output
                if len(pend) >= epdelay:
                    epilogue(*pend.pop(0))

                psA = psp.tile([P, F], f32, tag="psA")
                psB = psp.tile([P, F], f32, tag="psB")
                for c in range(CG):
                    if planes[c] == 0:
                        nc.tensor.matmul(psA[:], s_ap(c), g_t[:, c, :],
                                         start=(c == firstA), stop=(c == lastA))
                    else:
                        nc.tensor.matmul(psB[:], s_ap(c), g_t[:, c, :],
                                         start=(c == firstB), stop=(c == lastB))
                pend.append((psA, psB, g))
            for args in pend:
                epilogue(*args)

    nc.compile()
    return nc


def preprocess(vals, mEmbed, inter, row_idx, col_idx, tiles=TILES):
    E = row_idx.shape[0]
    col = col_idx.astype(np.int64) % MED
    rowl = row_idx.astype(np.int64)
    plane = rowl // MED
    prow = rowl % MED
    core = np.minimum(prow // RPC, NCORES - 1)
    lt = (prow - core * RPC) >> 7
    d = (prow & 127).astype(np.float32)

    # flexible-half assignment: cols in [FLEX_LO, H0_HI) may go to either
    # half; balance each (core, tile, plane) bucket.
    NB = NCORES * tiles * 2
    bucket = (core * tiles + lt) * 2 + plane
    rigid1 = col >= H0_HI
    flex = (col >= FLEX_LO) & ~rigid1
    n_r0 = np.bincount(bucket[(~rigid1) & (~flex)], minlength=NB)
    n_r1 = np.bincount(bucket[rigid1], minlength=NB)
    n_f = np.bincount(bucket[flex], minlength=NB)
    quota0 = np.clip((n_r0 + n_r1 + n_f + 1) // 2 - n_r0, 0, n_f)
    fi = np.where(flex)[0]
    fb = bucket[fi]
    fo = np.argsort(fb, kind="stable")
    fstarts = np.concatenate([[0], np.cumsum(np.bincount(fb, minlength=NB))[:-1]])
    frank = np.empty(len(fi), np.int64)
    frank[fo] = np.arange(len(fi)) - fstarts[fb[fo]]
    half = rigid1.astype(np.int64)
    half[fi[frank >= quota0[fb]]] = 1
    lidx = (col - half * FLEX_LO).astype(np.int16)

    run = half * 2 + plane                      # A0,B0,A1,B1 order
    key = (core * tiles + lt) * 4 + run
    order = np.argsort(key, kind="stable")
    ksort = key[order]
    nk = NCORES * tiles * 4
    cnt = np.bincount(ksort, minlength=nk)
    starts = np.concatenate([[0], np.cumsum(cnt)[:-1]])
    rank = np.arange(E, dtype=np.int64) - starts[ksort]

    cnt4 = cnt.reshape(-1, 4)
    C0 = max(1, int(np.ceil(cnt4[:, 0:2].max() / 128)))
    C1 = max(1, int(np.ceil(cnt4[:, 2:4].max() / 128)))
    CG = 2 * C0 + 2 * C1
    run_off = np.array([0, C0 * 128, 2 * C0 * 128, (2 * C0 + C1) * 128])
    SLOTS_G = CG * 128
    gidx = ksort // 4
    slot = gidx * SLOTS_G + run_off[ksort % 4] + rank
    TOT = NCORES * tiles * SLOTS_G

    IDX = np.zeros(TOT, np.int16)
    VAL = np.zeros(TOT, np.float32)
    DD = np.zeros(TOT, np.float32)
    IDX[slot] = lidx[order]
    VAL[slot] = np.asarray(vals, np.float32)[order]
    DD[slot] = d[order]

    # idx layout per chunk: 128 idxs wrapped into 16 partitions -> [16, 8],
    # replicated to 128 partitions; chunks side by side -> [128, CG*8]
    IDX5 = IDX.reshape(NCORES, tiles, CG, 8, 16).transpose(0, 1, 4, 2, 3)
    idx16 = IDX5.reshape(NCORES, tiles, 16, CG * 8)
    idx128 = np.ascontiguousarray(np.tile(idx16, (1, 1, 8, 1)))

    D4 = DD.reshape(NCORES, tiles, CG, 128).transpose(0, 1, 3, 2)
    V4 = VAL.reshape(NCORES, tiles, CG, 128).transpose(0, 1, 3, 2)
    dval = np.ascontiguousarray(np.concatenate([D4, V4], axis=3), dtype=np.float16)

    table16 = np.asarray(mEmbed, np.float32).astype(np.float16)
    Wmax = max(C0, C1)
    iota = np.ascontiguousarray(np.broadcast_to(
        np.repeat(np.arange(128, dtype=np.float16), Wmax)[None, :],
        (128, 128 * Wmax)))
    a = 2.0 * np.float32(np.asarray(inter).reshape(-1)[0])
    b = np.float32(2.0) - a
    ab = np.ascontiguousarray(
        np.stack([np.full(128, a, np.float32), np.full(128, b, np.float32)], axis=1))
    return C0, C1, table16, iota, ab, idx128, dval


def _run(vals, mEmbed, inter, row_idx, col_idx, trace=False):
    C0, C1, table16, iota, ab, idx128, dval = preprocess(
        vals, mEmbed, inter, row_idx, col_idx)
    key = (C0, C1, 1, ())
    if key not in _NC_CACHE:
        _NC_CACHE[key] = build_nc(C0, C1)
    nc = _NC_CACHE[key]
    in_maps = [
        {"table": table16, "iota": iota, "ab": ab,
         "idx": idx128[k], "dval": dval[k]}
        for k in range(NCORES)
    ]
    res = run_bass_kernel_spmd(nc, in_maps, core_ids=list(range(NCORES)),
                               trace=trace)
    full = np.concatenate(
        [res.results[k]["out"].reshape(RPC, F) for k in range(NCORES)], axis=0)
    return np.ascontiguousarray(full[:MED]), res


def kernel(vals, mEmbed, inter, row_idx, col_idx):
    out, _ = _run(vals, mEmbed, inter, row_idx, col_idx, trace=False)
    return out


def _make_sharded(nc, donate=False):
    import jax
    from jax.sharding import Mesh, PartitionSpec
    from jax.experimental.shard_map import shard_map
    from concourse import bass2jax as b2j

    b2j.install_neuronx_cc_hook()
    partition_name = nc.partition_id_tensor.name if nc.partition_id_tensor else None
    in_names, out_names, out_avals, zero_outs = [], [], [], []
    for alloc in nc.m.functions[0].allocations:
        if not isinstance(alloc, mybir.MemoryLocationSet):
            continue
        name = alloc.memorylocations[0].name
        if alloc.kind == "ExternalInput":
            if name != partition_name:
                in_names.append(name)
        elif alloc.kind == "ExternalOutput":
            out_names.append(name)
            shape = tuple(alloc.tensor_shape)
            dtype = mybir.dt.np(alloc.dtype)
            out_avals.append(jax.core.ShapedArray(shape, dtype))
            zero_outs.append(np.zeros(shape, dtype))
    n_params = len(in_names)
    in_names = in_names + out_names
    if partition_name is not None:
        in_names = in_names + [partition_name]

    def _body(*args):
        operands = list(args)
        if partition_name is not None:
            operands.append(b2j.partition_id_tensor())
        outs = b2j._bass_exec_p.bind(
            *operands,
            out_avals=tuple(out_avals),
            in_names=tuple(in_names),
            out_names=tuple(out_names),
            lowering_input_output_aliases=(),
            sim_require_finite=True,
            sim_require_nnan=True,
            nc=nc,
        )
        return tuple(outs)

    devices = jax.devices()[:NCORES]
    mesh = Mesh(np.asarray(devices), ("core",))
    in_specs = (PartitionSpec("core"),) * (n_params + len(out_names))
    out_specs = (PartitionSpec("core"),) * len(out_names)
    kw = dict(donate_argnums=tuple(range(n_params, n_params + len(out_names)))) if donate else {}

    sharded = jax.jit(
        shard_map(_body, mesh=mesh, in_specs=in_specs,
                  out_specs=out_specs, check_rep=False),
        keep_unused=True, **kw)
    return sharded, mesh, in_names[:n_params], out_names, zero_outs


def timed_run(vals, mEmbed, inter, row_idx, col_idx, k=4, samples=5,
              build_kwargs=None):
    """Time on device: build the same program with the body repeated 1x and
    kx INSIDE the NEFF; marginal = (T(k) - T(1)) / (k-1) = pure HW time."""
    import time
    import jax
    from jax.sharding import NamedSharding, PartitionSpec

    C0, C1, table16, iota, ab, idx128, dval = preprocess(
        vals, mEmbed, inter, row_idx, col_idx)
    bk = dict(build_kwargs or {})
    per_core = [
        {"table": table16, "iota": iota, "ab": ab,
         "idx": idx128[k_], "dval": dval[k_]}
        for k_ in range(NCORES)
    ]

    def measure(repeat):
        ck = (C0, C1, repeat, tuple(sorted(bk.items())))
        if ck not in _NC_CACHE:
            _NC_CACHE[ck] = build_nc(C0, C1, repeat=repeat, **bk)
        nc = _NC_CACHE[ck]
        sharded, mesh, in_names, out_names, zero_outs = _make_sharded(nc)
        sh = NamedSharding(mesh, PartitionSpec("core"))
        concat_in = [
            jax.device_put(
                np.concatenate([np.asarray(per_core[c][n]) for c in range(NCORES)],
                               axis=0), sh)
            for n in in_names
        ]
        concat_zero = [
            jax.device_put(np.zeros((NCORES * z.shape[0], *z.shape[1:]), z.dtype), sh)
            for z in zero_outs
        ]
        out = sharded(*concat_in, *concat_zero)
        jax.block_until_ready(out)
        best = float("inf")
        for _ in range(samples):
            t0 = time.perf_counter()
            out = sharded(*concat_in, *concat_zero)
            jax.block_until_ready(out)
            best = min(best, time.perf_counter() - t0)
        return best

    t1 = measure(1)
    tk = measure(k)
    marginal_ns = (tk - t1) / (k - 1) * 1e9
    return int(marginal_ns), int(t1 * 1e9), int(tk * 1e9)
